# revision 3
# baseline (speedup 1.0000x reference)
"""Trainium2 Bass kernel for nn_CausalTransformer (encoder-decoder, E=768 H=8
Dh=96 F=2048 V=30522 L=6+6 S=512 B=2).

Sharding: sequence-parallel. Core c handles batch row c//4, token chunk c%4
(128 tokens). All matmuls local; per attention layer one AllGather of the
per-chunk K/V projections across the 4-core row group. Cross-attn K/V for all
decoder layers are projected from enc_out chunks and AllGathered after the
encoder. Final LN output is AllGathered across all 8 cores and the LM head is
vocab-sharded (30528/8 = 3816 cols per core, padding trimmed on host).
Heads are zero-padded 96->128 so per-head slices align to partitions.
"""
import os
import sys

for _p in ("/opt/trn_rl_repo", "/root/.axon_site/_ro/trn_rl_repo"):
    if os.path.isdir(_p) and _p not in sys.path:
        sys.path.insert(0, _p)

import math
import numpy as np
import ml_dtypes

import concourse.bass as bass
import concourse.tile as tile
import concourse.mybir as mybir
from concourse.bass_utils import run_bass_kernel_spmd
from concourse.masks import make_identity

E, H, DH, F, V, SEQ, BB = 768, 8, 96, 2048, 30522, 512, 2
DP = 128          # padded head dim
EC = E // 128     # 6 e-chunks
FC = F // 128     # 16 f-chunks
T = 128           # tokens per core
NCHUNK = 4        # chunks per row
VPAD = 30528
VS = VPAD // 8    # vocab slice per core = 3816
LM_NT = [512, 512, 512, 512, 512, 512, 512, 232]  # N-tiles covering 3816

# matmul dtype knob: "float32" | "float32r" | "bfloat16"
MM_DT_NAME = os.environ.get("KERNEL_MM_DT", "bfloat16")

_BUILD_CACHE = {}


def _mm_dt():
    return getattr(mybir.dt, MM_DT_NAME)


def _mm_np():
    return {"float32": np.float32, "float32r": np.float32,
            "bfloat16": ml_dtypes.bfloat16}[MM_DT_NAME]


def _pos_embedding():
    pos = np.arange(SEQ)[:, None].astype(np.float32)
    div = np.exp(-np.arange(0, E, 2) * math.log(10000.0) / E).astype(np.float32)
    pe = np.empty((SEQ, E), dtype=np.float32)
    pe[:, 0::2] = np.sin(pos * div)
    pe[:, 1::2] = np.cos(pos * div)
    return pe


def _np(x):
    return np.asarray(x)


def _pack_qkv(w, gamma=None, scale=1.0):
    """w: [H, DH, E] per-head stack -> wT [EC, 128, H*DP] padded, f32.
    gamma: optional LN fold (multiply input features)."""
    w = _np(w).astype(np.float32) * scale
    if gamma is not None:
        w = w * _np(gamma).astype(np.float32)[None, None, :]
    out = np.zeros((E, H * DP), dtype=np.float32)
    for h in range(H):
        out[:, h * DP:h * DP + DH] = w[h].T  # [E, DH]
    return out.reshape(EC, 128, H * DP)


def _pack_wo(w):
    """w: [E, E] (out = attn_cat @ w.T) -> woT [H*DP-part layout] [H,128,E]->
    returned as [H, 128, E] f32 with padded rows zero."""
    w = _np(w).astype(np.float32)
    out = np.zeros((H, DP, E), dtype=np.float32)
    for h in range(H):
        out[h, :DH, :] = w[:, h * DH:(h + 1) * DH].T  # [DH, E]
    return out


def _pack_w1(w, gamma=None):
    """w: [F, E] -> w1T [EC, 128, F] f32."""
    w = _np(w).astype(np.float32)
    if gamma is not None:
        w = w * _np(gamma).astype(np.float32)[None, :]
    return w.T.reshape(EC, 128, F).copy()


def _pack_w2(w):
    """w: [E, F] -> w2T [FC, 128, E] f32."""
    return _np(w).astype(np.float32).T.reshape(FC, 128, E).copy()


def _ln_fold_bias(w_packed_T, beta):
    """bias contribution W @ beta for a packed [*, 128, O] weight, beta [E]."""
    flat = w_packed_T.reshape(E, -1)
    return flat.T @ _np(beta).astype(np.float32)


def _is_zero(v):
    return not np.any(_np(v))


def _is_identity_ln(p):
    return (not np.any(_np(p["b"]))) and np.all(_np(p["g"]) == 1.0)


class Cfg:
    def __init__(self, n_enc, n_dec, mm_dt_name):
        self.n_enc = n_enc
        self.n_dec = n_dec
        self.mm_dt_name = mm_dt_name

    def key(self):
        return (self.n_enc, self.n_dec, self.mm_dt_name)


# ---------------------------------------------------------------------------
# fixup: this walrus accepts at most 1 sync-wait per instruction; hoist excess
# waits onto prepended same-engine InstNoOp carriers.
def _split_fat_waits(nc):
    for b in nc.m.functions[0].blocks:
        insts = b.instructions
        i = 0
        while i < len(insts):
            inst = insts[i]
            si = getattr(inst, "sync_info", None)
            if si is not None and len(si.on_wait) > 1:
                waits = list(si.on_wait)
                ups = list(si.on_update)
                extra, keep = waits[:-1], waits[-1:]
                for k, w in enumerate(extra):
                    nd = mybir.InstNoOp(name=f"{inst.name}-wsplit{k}",
                                        ins=[], outs=[])
                    nd.engine = inst.engine
                    nd.sync_info = mybir.SyncInfo(on_wait=[w], on_update=[])
                    insts.insert(i, nd)
                    i += 1
                inst.sync_info = mybir.SyncInfo(on_wait=keep, on_update=ups)
            i += 1
    return nc


# ---------------------------------------------------------------------------
def _build(cfg: Cfg):
    """Builds the SPMD bass program. Returns (nc, input_names)."""
    mdt = getattr(mybir.dt, cfg.mm_dt_name)
    f32 = mybir.dt.float32
    nc = bass.Bass(num_devices=8)
    names = []

    def din(name, shape, dtype=None):
        names.append(name)
        return nc.dram_tensor(name, list(shape), dtype or mdt,
                              kind="ExternalInput")

    x0 = din("x0", [T, E], f32)
    y0 = din("y0", [T, E], f32)
    mask01 = din("mask01", [T, SEQ], f32)
    enc_w = []
    for l in range(cfg.n_enc):
        enc_w.append(dict(
            wq=din(f"e{l}_wq", [EC, 128, H * DP]),
            wk=din(f"e{l}_wk", [EC, 128, H * DP]),
            wv=din(f"e{l}_wv", [EC, 128, H * DP]),
            wo=din(f"e{l}_wo", [H, 128, E]),
            w1=din(f"e{l}_w1", [EC, 128, F]),
            w2=din(f"e{l}_w2", [FC, 128, E]),
        ))
    dec_w = []
    for l in range(cfg.n_dec):
        dec_w.append(dict(
            swq=din(f"d{l}_swq", [EC, 128, H * DP]),
            swk=din(f"d{l}_swk", [EC, 128, H * DP]),
            swv=din(f"d{l}_swv", [EC, 128, H * DP]),
            swo=din(f"d{l}_swo", [H, 128, E]),
            cwq=din(f"d{l}_cwq", [EC, 128, H * DP]),
            cwk=din(f"d{l}_cwk", [EC, 128, H * DP]),
            cwv=din(f"d{l}_cwv", [EC, 128, H * DP]),
            cwo=din(f"d{l}_cwo", [H, 128, E]),
            w1=din(f"d{l}_w1", [EC, 128, F]),
            w2=din(f"d{l}_w2", [FC, 128, E]),
        ))
    wlm = din("wlm", [EC, 128, VS])
    out = nc.dram_tensor("logits", [8, T, VS], f32, kind="ExternalOutput")

    RG4 = [[0, 1, 2, 3], [4, 5, 6, 7]]
    RG8 = [[0, 1, 2, 3, 4, 5, 6, 7]]

    with tile.TileContext(nc) as tc:
        _emit(nc, tc, cfg, mdt, x0, y0, mask01, enc_w, dec_w, wlm, out,
              RG4, RG8)

    _split_fat_waits(nc)
    return nc, names


def _emit(nc, tc, cfg, mdt, x0, y0, mask01, enc_w, dec_w, wlm, out, RG4, RG8):
    f32 = mybir.dt.float32
    from contextlib import ExitStack
    ctx = ExitStack()
    with ctx:
        singles = ctx.enter_context(tc.tile_pool(name="singles", bufs=1))
        actp = ctx.enter_context(tc.tile_pool(name="actp", bufs=2))
        wp = ctx.enter_context(tc.tile_pool(name="wp", bufs=2))
        kvp = ctx.enter_context(tc.tile_pool(name="kvp", bufs=2))
        smax = ctx.enter_context(tc.tile_pool(name="smax", bufs=3))
        evp = ctx.enter_context(tc.tile_pool(name="evp", bufs=3))
        psA = ctx.enter_context(tc.tile_pool(name="psA", bufs=2, space="PSUM"))
        psB = ctx.enter_context(tc.tile_pool(name="psB", bufs=2, space="PSUM"))
        psC = ctx.enter_context(tc.tile_pool(name="psC", bufs=2, space="PSUM"))
        dram = ctx.enter_context(tc.tile_pool(name="dram", bufs=1,
                                              space="DRAM"))

        ident = singles.tile([128, 128], mdt)
        make_identity(nc, ident)
        eps_t = singles.tile([128, 1], f32)
        nc.vector.memset(eps_t, 1e-5)
        mask_sb = singles.tile([T, SEQ], f32)
        nc.sync.dma_start(out=mask_sb, in_=mask01[:, :])

        # ---------------- primitives ----------------
        def transpose_act(x_sb, name):
            """token-major [128, E] (f32 or mdt) -> xT sbuf [128, EC, 128] mdt"""
            xT = actp.tile([128, EC, 128], mdt, name=f"{name}_xT")
            for c in range(EC):
                tp = psA.tile([128, 128], f32, name=f"{name}_tp", tag="tpose")
                nc.tensor.transpose(tp[:, :], x_sb[:, c * 128:(c + 1) * 128],
                                    ident)
                nc.vector.tensor_copy(out=xT[:, c, :], in_=tp[:, :])
            return xT

        def plain_ln(x_sb, name):
            """LayerNorm without gamma/beta. [128, E] f32 -> [128, E] f32."""
            stats = smax.tile([128, 3, 6], f32, name=f"{name}_st", tag="lnst")
            xg = x_sb.rearrange("p (g d) -> p g d", g=3)
            for g in range(3):
                nc.vector.bn_stats(out=stats[:, g, :], in_=xg[:, g, :])
            mv = smax.tile([128, 2], f32, name=f"{name}_mv", tag="lnmv")
            nc.vector.bn_aggr(out=mv[:, :], in_=stats[:, :, :])
            rstd = smax.tile([128, 1], f32, name=f"{name}_rs", tag="lnrs")
            nc.scalar.activation(out=rstd[:, :], in_=mv[:, 1:2],
                                 func=mybir.ActivationFunctionType.Sqrt,
                                 bias=eps_t[:, :])
            nc.vector.reciprocal(out=rstd[:, :], in_=rstd[:, :])
            o = actp.tile([128, E], f32, name=f"{name}_ln")
            nc.vector.tensor_scalar(out=o[:, :], in0=x_sb[:, :],
                                    scalar1=mv[:, 0:1], scalar2=rstd[:, :],
                                    op0=mybir.AluOpType.subtract,
                                    op1=mybir.AluOpType.mult)
            return o

        def proj_heads(xT, w_sb, name):
            """per-head feature-major projection: out [128(dp), H, T] mdt.
            w_sb [128, EC, H*DP]."""
            o = actp.tile([128, H, T], mdt, name=f"{name}_pT")
            for h in range(H):
                ps = psB.tile([128, T], f32, name=f"{name}_ps", tag="projh")
                for c in range(EC):
                    nc.tensor.matmul(ps[:, :],
                                     w_sb[:, c, h * DP:(h + 1) * DP],
                                     xT[:, c, :],
                                     start=(c == 0), stop=(c == EC - 1))
                nc.vector.tensor_copy(out=o[:, h, :], in_=ps[:, :])
            return o

        def proj_tokmajor(xT, w_sb, name):
            """token-major projection out [128(t), H*DP] mdt (for V)."""
            o = actp.tile([128, H * DP], mdt, name=f"{name}_v")
            for half in range(2):
                ps = psB.tile([128, 512], f32, name=f"{name}_psv", tag="projv")
                for c in range(EC):
                    nc.tensor.matmul(ps[:, :], xT[:, c, :],
                                     w_sb[:, c, half * 512:(half + 1) * 512],
                                     start=(c == 0), stop=(c == EC - 1))
                nc.vector.tensor_copy(out=o[:, half * 512:(half + 1) * 512],
                                      in_=ps[:, :])
            return o

        def load_w(wdram, shape, name, rearr):
            w_sb = wp.tile(shape, mdt, name=name, tag=name.split("_", 1)[-1])
            nc.sync.dma_start(out=w_sb, in_=wdram[:].rearrange(rearr))
            return w_sb

        def kv_project_and_ag(xT, wk_sb, wv_sb, name):
            """project own-chunk K^T and V, bounce to DRAM, AllGather(4)."""
            kT = proj_heads(xT, wk_sb, f"{name}_k")
            v = proj_tokmajor(xT, wv_sb, f"{name}_v")
            ag_in = dram.tile([2, H, 128, 128], mdt, name=f"{name}_agi",
                              tag="agi")
            ag_out = dram.tile([NCHUNK, 2, H, 128, 128], mdt,
                               name=f"{name}_ago", tag="ago")
            nc.sync.dma_start(out=ag_in[0].rearrange("h d t -> d h t"),
                              in_=kT[:, :, :])
            nc.sync.dma_start(
                out=ag_in[1].rearrange("h t d -> t (h d)"), in_=v[:, :])
            nc.gpsimd.collective_compute(
                "AllGather", mybir.AluOpType.bypass, replica_groups=RG4,
                ins=[ag_in[:].opt()], outs=[ag_out[:].opt()])
            return ag_out

        def load_kv_full(ag_out, name):
            kT_f = kvp.tile([128, H, NCHUNK, 128], mdt, name=f"{name}_kf",
                            tag="kf")
            for h in range(H):
                nc.sync.dma_start(
                    out=kT_f[:, h, :, :],
                    in_=ag_out[:, 0, h].rearrange("r d t -> d r t"))
            v_f = kvp.tile([128, NCHUNK, H, 128], mdt, name=f"{name}_vf",
                           tag="vf")
            for r in range(NCHUNK):
                nc.sync.dma_start(
                    out=v_f[:, r, :, :],
                    in_=ag_out[r, 1].rearrange("h t d -> t h d"))
            return kT_f, v_f

        def attention(x_in_sb, qT_src_xT, kv, w_wo_sb, causal, name):
            """kv = (kT_f, v_f); returns attn output token-major [128,E] f32
            in PSUM-evacuated sbuf (no residual added here)."""
            kT_f, v_f = kv
            attnT = actp.tile([128, H, T], mdt, name=f"{name}_aT")
            for h in range(H):
                sc = psB.tile([T, SEQ], f32, name=f"{name}_sc", tag="score")
                nc.tensor.matmul(sc[:, :], qT_src_xT[:, h, :],
                                 kT_f[:, h, :, :], start=True, stop=True)
                pexp = smax.tile([T, SEQ], f32, name=f"{name}_pe", tag="pexp")
                denom = smax.tile([T, 1], f32, name=f"{name}_dn", tag="denom")
                if causal:
                    nc.scalar.activation(out=pexp[:, :], in_=sc[:, :],
                                         func=mybir.ActivationFunctionType.Exp)
                    nc.vector.tensor_mul(out=pexp[:, :], in0=pexp[:, :],
                                         in1=mask_sb[:, :])
                    nc.vector.reduce_sum(out=denom[:, :], in_=pexp[:, :],
                                         axis=mybir.AxisListType.X)
                else:
                    nc.scalar.activation(out=pexp[:, :], in_=sc[:, :],
                                         func=mybir.ActivationFunctionType.Exp,
                                         accum_out=denom[:, :])
                nc.vector.reciprocal(out=denom[:, :], in_=denom[:, :])
                P = smax.tile([T, SEQ], mdt, name=f"{name}_P", tag="pnorm")
                nc.vector.tensor_scalar_mul(out=P[:, :], in0=pexp[:, :],
                                            scalar1=denom[:, :])
                # transpose P chunks -> PT [128(k), NCHUNK, T]
                PT = smax.tile([128, NCHUNK, T], mdt, name=f"{name}_PT",
                               tag="ptrans")
                for r in range(NCHUNK):
                    tp = psA.tile([128, T], f32, name=f"{name}_ptp",
                                  tag="tpose")
                    nc.tensor.transpose(tp[:, :], P[:, r * 128:(r + 1) * 128],
                                        ident)
                    nc.vector.tensor_copy(out=PT[:, r, :], in_=tp[:, :])
                av = psC.tile([128, T], f32, name=f"{name}_av", tag="av")
                for r in range(NCHUNK):
                    nc.tensor.matmul(av[:, :], v_f[:, r, h, :], PT[:, r, :],
                                     start=(r == 0), stop=(r == NCHUNK - 1))
                nc.vector.tensor_copy(out=attnT[:, h, :], in_=av[:, :])
            # Wo: out token-major [T, E] f32, two N-halves
            o = evp.tile([T, E], f32, name=f"{name}_wo")
            for half in range(2):
                ps = psB.tile([T, 384], f32, name=f"{name}_wops", tag="wo")
                for h in range(H):
                    nc.tensor.matmul(ps[:, :], attnT[:, h, :],
                                     w_wo_sb[:, h, half * 384:(half + 1) * 384],
                                     start=(h == 0), stop=(h == H - 1))
                nc.vector.tensor_copy(out=o[:, half * 384:(half + 1) * 384],
                                      in_=ps[:, :])
            return o

        def ffn(h_sb, w1_sb, w2_sb, name):
            hT = transpose_act(h_sb, f"{name}_h")
            h1T = actp.tile([128, FC, T], mdt, name=f"{name}_h1T")
            for fb in range(FC):
                ps = psB.tile([128, T], f32, name=f"{name}_h1ps", tag="h1")
                for c in range(EC):
                    nc.tensor.matmul(ps[:, :],
                                     w1_sb[:, c, fb * 128:(fb + 1) * 128],
                                     hT[:, c, :],
                                     start=(c == 0), stop=(c == EC - 1))
                nc.scalar.activation(out=h1T[:, fb, :], in_=ps[:, :],
                                     func=mybir.ActivationFunctionType.Gelu)
            o = evp.tile([T, E], f32, name=f"{name}_f2")
            for half in range(2):
                ps = psB.tile([T, 384], f32, name=f"{name}_w2ps", tag="w2")
                for fb in range(FC):
                    nc.tensor.matmul(ps[:, :], h1T[:, fb, :],
                                     w2_sb[:, fb, half * 384:(half + 1) * 384],
                                     start=(fb == 0), stop=(fb == FC - 1))
                nc.vector.tensor_copy(out=o[:, half * 384:(half + 1) * 384],
                                      in_=ps[:, :])
            return o

        def residual_add(a, b, name):
            o = actp.tile([T, E], f32, name=f"{name}_res")
            nc.vector.tensor_add(out=o[:, :], in0=a[:, :], in1=b[:, :])
            return o

        # ---------------- encoder ----------------
        x = actp.tile([T, E], f32, name="x_in")
        nc.sync.dma_start(out=x, in_=x0[:, :])
        for l, w in enumerate(enc_w):
            nm = f"e{l}"
            wq_sb = load_w(w["wq"], [128, EC, H * DP], f"{nm}_wq",
                           "c p d -> p c d")
            wk_sb = load_w(w["wk"], [128, EC, H * DP], f"{nm}_wk",
                           "c p d -> p c d")
            wv_sb = load_w(w["wv"], [128, EC, H * DP], f"{nm}_wv",
                           "c p d -> p c d")
            wo_sb = load_w(w["wo"], [128, H, E], f"{nm}_wo", "h p e -> p h e")
            xT = transpose_act(x, nm)
            ag = kv_project_and_ag(xT, wk_sb, wv_sb, nm)
            qT = proj_heads(xT, wq_sb, f"{nm}_q")
            kv = load_kv_full(ag, nm)
            attn = attention(x, qT, kv, wo_sb, False, nm)
            h = plain_ln(residual_add(x, attn, f"{nm}_r1"), f"{nm}_ln1")
            w1_sb = load_w(w["w1"], [128, EC, F], f"{nm}_w1", "c p f -> p c f")
            w2_sb = load_w(w["w2"], [128, FC, E], f"{nm}_w2", "c p e -> p c e")
            ff = ffn(h, w1_sb, w2_sb, nm)
            x = plain_ln(residual_add(ff, h, f"{nm}_r2"), f"{nm}_ln2")

        enc_out = x

        # ---------------- cross K/V precompute ----------------
        cross_kv = []
        if cfg.n_dec:
            eT = transpose_act(enc_out, "eo")
            for l, w in enumerate(dec_w):
                nm = f"c{l}"
                wk_sb = load_w(w["cwk"], [128, EC, H * DP], f"{nm}_cwk",
                               "c p d -> p c d")
                wv_sb = load_w(w["cwv"], [128, EC, H * DP], f"{nm}_cwv",
                               "c p d -> p c d")
                cross_kv.append(kv_project_and_ag(eT, wk_sb, wv_sb, nm))

        # ---------------- decoder ----------------
        y = actp.tile([T, E], f32, name="y_in")
        nc.sync.dma_start(out=y, in_=y0[:, :])
        for l, w in enumerate(dec_w):
            nm = f"d{l}"
            ny = plain_ln(y, f"{nm}_ln1")
            nyT = transpose_act(ny, f"{nm}_ny")
            swq_sb = load_w(w["swq"], [128, EC, H * DP], f"{nm}_swq",
                            "c p d -> p c d")
            swk_sb = load_w(w["swk"], [128, EC, H * DP], f"{nm}_swk",
                            "c p d -> p c d")
            swv_sb = load_w(w["swv"], [128, EC, H * DP], f"{nm}_swv",
                            "c p d -> p c d")
            swo_sb = load_w(w["swo"], [128, H, E], f"{nm}_swo",
                            "h p e -> p h e")
            ag = kv_project_and_ag(nyT, swk_sb, swv_sb, nm)
            qT = proj_heads(nyT, swq_sb, f"{nm}_sq")
            kv = load_kv_full(ag, nm)
            sattn = attention(ny, qT, kv, swo_sb, True, f"{nm}_s")
            y = residual_add(y, sattn, f"{nm}_rs")

            ny2 = plain_ln(y, f"{nm}_ln2")
            ny2T = transpose_act(ny2, f"{nm}_ny2")
            cwq_sb = load_w(w["cwq"], [128, EC, H * DP], f"{nm}_cwq",
                            "c p d -> p c d")
            cwo_sb = load_w(w["cwo"], [128, H, E], f"{nm}_cwo",
                            "h p e -> p h e")
            qT2 = proj_heads(ny2T, cwq_sb, f"{nm}_cq")
            ckv = load_kv_full(cross_kv[l], f"{nm}_c")
            cattn = attention(ny2, qT2, ckv, cwo_sb, False, f"{nm}_c")
            y = residual_add(y, cattn, f"{nm}_rc")

            ny3 = plain_ln(y, f"{nm}_ln3")
            w1_sb = load_w(w["w1"], [128, EC, F], f"{nm}_w1", "c p f -> p c f")
            w2_sb = load_w(w["w2"], [128, FC, E], f"{nm}_w2", "c p e -> p c e")
            ff = ffn(ny3, w1_sb, w2_sb, nm)
            y = residual_add(y, ff, f"{nm}_rf")

        # ---------------- final LN + AllGather + LM head ----------------
        yf = plain_ln(y, "lnf")
        yfT = transpose_act(yf, "yf")
        agf_in = dram.tile([EC, 128, 128], mdt, name="agf_in")
        nc.sync.dma_start(out=agf_in[:].rearrange("c p t -> p c t"), in_=yfT)
        agf_out = dram.tile([8, EC, 128, 128], mdt, name="agf_out")
        nc.gpsimd.collective_compute(
            "AllGather", mybir.AluOpType.bypass, replica_groups=RG8,
            ins=[agf_in[:].opt()], outs=[agf_out[:].opt()])
        yT_full = kvp.tile([128, EC, 8, 128], mdt, name="yT_full")
        for c in range(EC):
            nc.sync.dma_start(out=yT_full[:, c, :, :],
                              in_=agf_out[:, c].rearrange("r p t -> p r t"))
        off = 0
        for ni, w in enumerate(LM_NT):
            wlm_sb = wp.tile([128, EC, 512], mdt, name=f"lm{ni}_w", tag="lmw")
            nc.sync.dma_start(out=wlm_sb[:, :, :w],
                              in_=wlm[:, :, off:off + w].rearrange(
                                  "c p v -> p c v"))
            for r in range(8):
                ps = psB.tile([128, 512], f32, name=f"lm{ni}_{r}ps", tag="lmps")
                for c in range(EC):
                    nc.tensor.matmul(ps[:, :w], yT_full[:, c, r, :],
                                     wlm_sb[:, c, :w],
                                     start=(c == 0), stop=(c == EC - 1))
                ev = evp.tile([128, 512], f32, name=f"lm{ni}_{r}ev", tag="lmev")
                nc.vector.tensor_copy(out=ev[:, :w], in_=ps[:, :w])
                nc.sync.dma_start(out=out[r, :, off:off + w], in_=ev[:, :w])
            off += w


# ---------------------------------------------------------------------------
def _pack_inputs(src, trg, params):
    """Host-side: embeddings, weight packing/folding, per-core input maps."""
    src = _np(src)
    trg = _np(trg)
    pe = _pos_embedding()
    emb_s = _np(params["emb_src"]).astype(np.float32)
    emb_t = _np(params["emb_trg"]).astype(np.float32)
    x0 = emb_s[src] + pe[None, :, :]   # [B, S, E]
    y0 = emb_t[trg] + pe[None, :, :]

    n_enc = len(params["enc"])
    n_dec = len(params["dec"])
    mmnp = _mm_np()

    shared = {}

    def put(name, arr):
        shared[name] = np.ascontiguousarray(arr.astype(mmnp))

    scale = 1.0 / math.sqrt(DH)
    for l, lp in enumerate(params["enc"]):
        a = lp["attn"]
        assert _is_zero(a["bo"]), "nonzero bo unsupported"
        assert _is_identity_ln(lp["ln1"]) and _is_identity_ln(lp["ln2"]), \
            "non-identity encoder LN unsupported"
        f = lp["ff"]
        assert _is_zero(f["b1"]) and _is_zero(f["b2"])
        put(f"e{l}_wq", _pack_qkv(a["Wq"], scale=scale))
        put(f"e{l}_wk", _pack_qkv(a["Wk"]))
        put(f"e{l}_wv", _pack_qkv(a["Wv"]))
        put(f"e{l}_wo", _pack_wo(a["Wo"]))
        put(f"e{l}_w1", _pack_w1(f["W1"]))
        put(f"e{l}_w2", _pack_w2(f["W2"]))
    for l, lp in enumerate(params["dec"]):
        s, c, f = lp["self"], lp["cross"], lp["ff"]
        for p in (lp["ln1"], lp["ln2"], lp["ln3"]):
            assert _is_identity_ln(p), "non-identity decoder LN unsupported"
        assert _is_zero(s["bo"]) and _is_zero(c["bo"])
        assert _is_zero(f["b1"]) and _is_zero(f["b2"])
        put(f"d{l}_swq", _pack_qkv(s["Wq"], scale=scale))
        put(f"d{l}_swk", _pack_qkv(s["Wk"]))
        put(f"d{l}_swv", _pack_qkv(s["Wv"]))
        put(f"d{l}_swo", _pack_wo(s["Wo"]))
        put(f"d{l}_cwq", _pack_qkv(c["Wq"], scale=scale))
        put(f"d{l}_cwk", _pack_qkv(c["Wk"]))
        put(f"d{l}_cwv", _pack_qkv(c["Wv"]))
        put(f"d{l}_cwo", _pack_wo(c["Wo"]))
        put(f"d{l}_w1", _pack_w1(f["W1"]))
        put(f"d{l}_w2", _pack_w2(f["W2"]))
    assert _is_identity_ln(params["ln_f"]), "non-identity ln_f unsupported"
    wlm_full = _np(params["Wlm"]).astype(np.float32)  # [V, E]
    wlm_pad = np.zeros((VPAD, E), dtype=np.float32)
    wlm_pad[:V] = wlm_full

    in_maps = []
    for core in range(8):
        b, q = core // NCHUNK, core % NCHUNK
        m = dict(shared)
        m["x0"] = np.ascontiguousarray(
            x0[b, q * T:(q + 1) * T].astype(np.float32))
        m["y0"] = np.ascontiguousarray(
            y0[b, q * T:(q + 1) * T].astype(np.float32))
        kidx = np.arange(SEQ)[None, :]
        tidx = q * T + np.arange(T)[:, None]
        m["mask01"] = (kidx <= tidx).astype(np.float32)
        vsl = wlm_pad[core * VS:(core + 1) * VS]  # [VS, E]
        m["wlm"] = np.ascontiguousarray(
            vsl.T.reshape(EC, 128, VS).astype(mmnp))
        in_maps.append(m)
    return in_maps, (n_enc, n_dec)


def _get_built(n_enc, n_dec):
    cfg = Cfg(n_enc, n_dec, MM_DT_NAME)
    k = cfg.key()
    if k not in _BUILD_CACHE:
        _BUILD_CACHE[k] = _build(cfg)
    return _BUILD_CACHE[k]


def kernel(src, trg, params):
    in_maps, (n_enc, n_dec) = _pack_inputs(src, trg, params)
    nc, names = _get_built(n_enc, n_dec)
    res = run_bass_kernel_spmd(nc, in_maps, core_ids=list(range(8)))
    # assemble: core c -> logits[:, :, c*VS:(c+1)*VS]; out[r] = token block r
    full = np.empty((BB, SEQ, VPAD), dtype=np.float32)
    for core in range(8):
        o = res.results[core]["logits"]  # [8, T, VS]
        toks = o.reshape(8 * T, VS)      # ranks 0..7 = token blocks
        full[:, :, core * VS:(core + 1) * VS] = toks.reshape(BB, SEQ, VS)
    logits = full[:, :, :V] + _np(params["blm"]).astype(np.float32)
    return logits.astype(np.float32)
